# revision 4
# baseline (speedup 1.0000x reference)
"""BertSelfAttention TRN2 kernel: B=4, S=2048, H=1024, NH=16, HD=64, 8 cores.

Sharding: core c -> batch b = c // 2, head group g = c % 2 (heads g*8 .. g*8+7).
Each core computes out[b, :, g*512:(g+1)*512].

Per-core device algorithm (all matmul operands bf16, fp32 PSUM accum):
  phase 1: qT/kT = WT.T @ hT  -> [512, 2048] (d on partitions, 4 head-pair tiles)
           v = hT.T @ WvT     -> v_aug [s, 8*65] with ones column per head
  phase 2 per head, per j-tile (key tile of 128):
           scoresT[j, i] = kT_h.T @ qT_h   (K=d=64), two [128,1024] psum tiles
           eT = exp(scoresT * 0.125 + mask_j)    (ACT, bf16 out)
           P_c[65, 512] += v_aug_h.T @ eT chunk  (accumulate over j; row 64 = sums)
  normalize: ctx = P_c[0:64] * (1/P_c[64]) + bv, DMA out transposed.

Host side: transposes/casts inputs (bf16), splits per core, assembles output.
The whole device body can be wrapped in a For_i(reps) loop for slope timing.
"""
from contextlib import ExitStack

import numpy as np
import ml_dtypes

B, S, H = 4, 2048, 1024
NH, HD = 16, 64
NCORES = 8
HPC = 8          # heads per core
DG = HPC * HD    # feature cols per core = 512
CT = H // 128    # c-tiles = 8
ST = S // 128    # s-tiles = 16
IC = S // 512    # i-chunks of 512 = 4

_cache = {}


def _build(reps=1):
    import concourse.bass as bass  # noqa: F401
    import concourse.mybir as mybir
    import concourse.tile as tile
    from concourse import bacc

    fp32 = mybir.dt.float32
    bf16 = mybir.dt.bfloat16
    EXP = mybir.ActivationFunctionType.Exp
    IDENT = mybir.ActivationFunctionType.Identity

    nc = bacc.Bacc()
    hT = nc.declare_dram_parameter("hT", [H, S], bf16, isOutput=False)
    wqT = nc.declare_dram_parameter("wqT", [H, DG], bf16, isOutput=False)
    wkT = nc.declare_dram_parameter("wkT", [H, DG], bf16, isOutput=False)
    wvT = nc.declare_dram_parameter("wvT", [H, DG], bf16, isOutput=False)
    bqp = nc.declare_dram_parameter("bq", [128, 4], fp32, isOutput=False)
    bkp = nc.declare_dram_parameter("bk", [128, 4], fp32, isOutput=False)
    bvp = nc.declare_dram_parameter("bv", [64, HPC], fp32, isOutput=False)
    maskp = nc.declare_dram_parameter("mask", [128, ST], fp32, isOutput=False)
    outp = nc.declare_dram_parameter("out", [S, DG], fp32, isOutput=True)

    with ExitStack() as ctx:
        tc = ctx.enter_context(tile.TileContext(nc))
        sb = ctx.enter_context(tc.tile_pool(name="sb", bufs=1))
        spool = ctx.enter_context(tc.tile_pool(name="spool", bufs=2, space="PSUM"))
        pvpool = ctx.enter_context(tc.tile_pool(name="pvpool", bufs=1, space="PSUM"))
        epool = ctx.enter_context(tc.tile_pool(name="epool", bufs=3))
        fin = ctx.enter_context(tc.tile_pool(name="fin", bufs=4))

        def body(_iv=None):
            # ---- loads ----
            qT = sb.tile([128, 4, S], bf16, name="qT", tag="qT")
            kT = sb.tile([128, 4, S], bf16, name="kT", tag="kT")
            v_aug = sb.tile([128, ST, HPC * 65], bf16, name="v_aug", tag="v_aug")
            bq_sb = sb.tile([128, 4], fp32, name="bq_sb", tag="bq_sb")
            bk_sb = sb.tile([128, 4], fp32, name="bk_sb", tag="bk_sb")
            bv_sb = sb.tile([64, HPC], fp32, name="bv_sb", tag="bv_sb")
            mask_sb = sb.tile([128, ST], fp32, name="mask_sb", tag="mask_sb")
            hT_sb = sb.tile([128, CT, S], bf16, name="hT_sb", tag="hT_sb")
            wq_sb = sb.tile([128, CT, DG], bf16, name="wq_sb", tag="wq_sb")
            wk_sb = sb.tile([128, CT, DG], bf16, name="wk_sb", tag="wk_sb")
            wv_sb = sb.tile([128, CT, DG], bf16, name="wv_sb", tag="wv_sb")

            nc.sync.dma_start(out=bq_sb, in_=bqp[:, :])
            nc.sync.dma_start(out=bk_sb, in_=bkp[:, :])
            nc.sync.dma_start(out=bv_sb, in_=bvp[:, :])
            nc.sync.dma_start(out=mask_sb, in_=maskp[:, :])
            nc.vector.memset(v_aug, 1.0)
            for t in range(CT):
                nc.sync.dma_start(
                    out=hT_sb[:, t, :], in_=hT[t * 128:(t + 1) * 128, :])
            nc.sync.dma_start(out=wq_sb, in_=wqT.rearrange("(t p) d -> p t d", p=128))
            nc.sync.dma_start(out=wk_sb, in_=wkT.rearrange("(t p) d -> p t d", p=128))
            nc.sync.dma_start(out=wv_sb, in_=wvT.rearrange("(t p) d -> p t d", p=128))

            # ---- phase 1: projections ----
            for (w_sb, dst, bias) in ((wq_sb, qT, bq_sb), (wk_sb, kT, bk_sb)):
                for dt_ in range(4):
                    for sc in range(IC):
                        ps = spool.tile([128, 512], fp32, name="pjq", tag="s_ps")
                        for ct in range(CT):
                            nc.tensor.matmul(
                                ps,
                                w_sb[:, ct, dt_ * 128:(dt_ + 1) * 128],
                                hT_sb[:, ct, sc * 512:(sc + 1) * 512],
                                start=(ct == 0), stop=(ct == CT - 1),
                            )
                        nc.scalar.activation(
                            dst[:, dt_, sc * 512:(sc + 1) * 512], ps, IDENT,
                            bias=bias[:, dt_:dt_ + 1], scale=1.0,
                        )
            for st in range(ST):
                ps = spool.tile([128, 512], fp32, name="pjv", tag="s_ps")
                for ct in range(CT):
                    nc.tensor.matmul(
                        ps,
                        hT_sb[:, ct, st * 128:(st + 1) * 128],
                        wv_sb[:, ct, :],
                        start=(ct == 0), stop=(ct == CT - 1),
                    )
                for h in range(HPC):
                    nc.vector.tensor_copy(
                        v_aug[:, st, h * 65:h * 65 + 64],
                        ps[:, h * 64:(h + 1) * 64],
                    )

            # ---- phase 2: attention ----
            for h in range(HPC):
                hp, lo = h // 2, (h % 2 == 0)
                r0, r1 = (0, 64) if lo else (64, 128)
                pcs = [pvpool.tile([65, 512], fp32, name=f"pc{c}", tag=f"pc{c}")
                       for c in range(IC)]
                for jt in range(ST):
                    for half in range(2):
                        s_ps = spool.tile([128, 1024], fp32, name="s_ps",
                                          tag="s_ps")
                        for cc in range(2):
                            c = half * 2 + cc
                            nc.tensor.matmul(
                                s_ps[:, cc * 512:(cc + 1) * 512],
                                kT[r0:r1, hp, jt * 128:(jt + 1) * 128],
                                qT[r0:r1, hp, c * 512:(c + 1) * 512],
                                start=True, stop=True,
                            )
                        eT = epool.tile([128, 1024], bf16, name="eT", tag="eT")
                        nc.scalar.activation(
                            eT, s_ps, EXP,
                            bias=mask_sb[:, jt:jt + 1], scale=0.125,
                        )
                        for cc in range(2):
                            c = half * 2 + cc
                            nc.tensor.matmul(
                                pcs[c],
                                v_aug[:, jt, h * 65:(h + 1) * 65],
                                eT[:, cc * 512:(cc + 1) * 512],
                                start=(jt == 0), stop=(jt == ST - 1),
                            )
                for c in range(IC):
                    rec = fin.tile([1, 512], fp32, name="rec", tag="rec")
                    nc.vector.reciprocal(rec, pcs[c][64:65, :])
                    rec_bc = fin.tile([64, 512], fp32, name="rec_bc", tag="rec_bc")
                    nc.gpsimd.partition_broadcast(rec_bc, rec)
                    stage = fin.tile([64, 512], fp32, name="stage", tag="stage")
                    nc.vector.tensor_mul(stage, pcs[c][0:64, :], rec_bc)
                    nc.vector.tensor_scalar_add(stage, stage, bv_sb[:, h:h + 1])
                    nc.sync.dma_start(
                        out=outp[c * 512:(c + 1) * 512,
                                 h * 64:(h + 1) * 64].rearrange("s d -> d s"),
                        in_=stage,
                    )

        if reps == 1:
            body()
        else:
            with tc.For_i(0, reps, 1) as iv:
                body(iv)

    nc.compile()
    return nc


def _prep_inputs(hidden_states, attention_mask, Wq, bq, Wk, bk, Wv, bv):
    bf = ml_dtypes.bfloat16
    hidden_states = np.asarray(hidden_states, dtype=np.float32)
    attention_mask = np.asarray(attention_mask, dtype=np.float32)
    Wq = np.asarray(Wq, dtype=np.float32)
    Wk = np.asarray(Wk, dtype=np.float32)
    Wv = np.asarray(Wv, dtype=np.float32)
    bq = np.asarray(bq, dtype=np.float32)
    bk = np.asarray(bk, dtype=np.float32)
    bv = np.asarray(bv, dtype=np.float32)

    hT = [np.ascontiguousarray(hidden_states[b].T).astype(bf) for b in range(B)]
    wT = {}
    for name, W in (("q", Wq), ("k", Wk), ("v", Wv)):
        for g in range(2):
            wT[(name, g)] = np.ascontiguousarray(
                W[g * DG:(g + 1) * DG, :].T).astype(bf)
    in_maps = []
    for c in range(NCORES):
        b, g = c // 2, c % 2
        bqg = bq[g * DG:(g + 1) * DG]
        bkg = bk[g * DG:(g + 1) * DG]
        bvg = bv[g * DG:(g + 1) * DG]
        mask_b = attention_mask[b, 0, 0, :]
        in_maps.append({
            "hT": hT[b],
            "wqT": wT[("q", g)], "wkT": wT[("k", g)], "wvT": wT[("v", g)],
            "bq": np.ascontiguousarray(bqg.reshape(4, 128).T),
            "bk": np.ascontiguousarray(bkg.reshape(4, 128).T),
            "bv": np.ascontiguousarray(bvg.reshape(HPC, 64).T),
            "mask": np.ascontiguousarray(mask_b.reshape(ST, 128).T),
        })
    return in_maps


def _run(in_maps, reps=1):
    from concourse.bass_utils import run_bass_kernel_spmd

    key = f"nc{reps}"
    if key not in _cache:
        _cache[key] = _build(reps)
    return run_bass_kernel_spmd(_cache[key], in_maps, list(range(NCORES)))


def kernel(hidden_states, attention_mask, Wq, bq, Wk, bk, Wv, bv):
    in_maps = _prep_inputs(
        hidden_states, attention_mask, Wq, bq, Wk, bk, Wv, bv)
    res = _run(in_maps)
    out = np.empty((B, S, H), dtype=np.float32)
    for c in range(NCORES):
        b, g = c // 2, c % 2
        out[b, :, g * DG:(g + 1) * DG] = res.results[c]["out"]
    return out
